# revision 6
# baseline (speedup 1.0000x reference)
# Trainium2 Bass kernel for nn_Conv_58394375356438:
# single-input-channel 7x7 conv, B=32, F=64, H=W=224, fp32.
#
# Data-parallel across 8 NeuronCores (4 images each). Per core, a
# "block-im2col" scheme: the host-padded image is expanded on-chip into
# SBUF tiles covering 16 rows x 7 horizontal shifts, laid out as four
# 28-partition blocks at partition bases {0,32,64,96}. Output rows are
# computed in pairs (r, r+half): one PSUM accumulation chain of 3-4
# K=28 matmuls (M=128 = 64 channels x 2 rows; tile_position row selects
# the 32-partition block). Weight tables (row-pair x block aligned,
# zero-padded) are precomputed on the host. PSUM is drained by
# VectorE/ScalarE into staging tiles shaped so each output DMA writes
# 128 partitions with multi-KB contiguous HBM runs.
import sys

sys.path.insert(0, "/opt/trn_rl_repo")

import numpy as np

import concourse.bacc as bacc
import concourse.bass as bass
import concourse.mybir as mybir
import concourse.tile as tile
from concourse.bass_utils import run_bass_kernel_spmd

N_CORES = 8
B_FULL, F, KS, PAD = 32, 64, 7, 3
H = W = 224
B_LOC = B_FULL // N_CORES          # images per core
B2 = 2                             # images packed per matmul N
WP = W + 2 * PAD                   # 230
XP_ROWS = 236                      # 224 + 6 pad + tile overrun
GROUP = 10                         # output rows per group
TILE_ROWS = 16
N_GROUPS = (H + GROUP - 1) // GROUP

f32 = mybir.dt.float32
f32r = mybir.dt.float32r
bf16 = mybir.dt.bfloat16

# operand dtype mode: "f32r" | "bf16" | "f32"
MODE = "f32r"

# ordered weight tables: (spacing, t0); table[32*blk+7*rho+dj, f+64*s] =
# kernel[f, t0 - spacing*s + rho, dj] (0 outside range)
TABLES = [(5, t) for t in range(-3, 12)] + [(2, t) for t in (-1, 0, 3, 4, 7, 8)]
TIDX = {st: i for i, st in enumerate(TABLES)}
NT = len(TABLES)

_cache = {}


def _make_weight_tables(kern):
    wtab = np.zeros((NT, 128, 128), np.float32)
    for i, (spacing, t0) in enumerate(TABLES):
        for s in range(2):
            t = t0 - spacing * s
            for rho in range(4):
                di = t + rho
                if 0 <= di < KS:
                    # rows 32*blk + 7*rho + dj for all blk, cols f + 64*s
                    for blk in range(4):
                        r0 = 32 * blk + 7 * rho
                        wtab[i, r0:r0 + KS, 64 * s:64 * s + F] = \
                            kern[:, di, :].T  # [dj, f]
    return wtab


def _build_program(mode):
    mmdt = {"f32r": f32r, "bf16": bf16, "f32": f32}[mode]

    nc = bacc.Bacc("TRN2", target_bir_lowering=False)

    xp = nc.dram_tensor("xp", [B_LOC, XP_ROWS, WP], mmdt, kind="ExternalInput")
    wtab_d = nc.dram_tensor("wtab", [NT, 128, 128], mmdt, kind="ExternalInput")
    out_d = nc.dram_tensor("out", [B_LOC, F, H, W], f32, kind="ExternalOutput")

    def xp_window_ap(b, row0):
        # overlapping read AP [rho:4][dj:7][c:224] at xp[b, row0, 0]
        return bass.AP(
            tensor=xp,
            offset=b * XP_ROWS * WP + row0 * WP,
            ap=[[WP, 4], [1, KS], [1, W]],
        )

    with tile.TileContext(nc) as tc:
        with tc.tile_pool(name="wt", bufs=1) as wtpool, \
             tc.tile_pool(name="bt", bufs=4) as btpool, \
             tc.tile_pool(name="stage", bufs=4) as stpool, \
             tc.tile_pool(name="psum", bufs=7, space="PSUM") as pspool:

            wts = []
            for i in range(NT):
                wt = wtpool.tile([28, 128], mmdt, tag=f"wt{i}")
                nc.sync.dma_start(wt[:], wtab_d[i, 0:28, :])
                wts.append(wt)

            for pair in range(B_LOC // B2):
                b0 = B2 * pair
                for q in range(N_GROUPS):
                    g = GROUP * q
                    nrows = min(GROUP, H - g)
                    half = nrows // 2
                    bts = []
                    for blk in range(4):
                        btk = btpool.tile([28, B2 * W], mmdt, tag=f"bt{blk}")
                        for bi in range(B2):
                            nc.sync.dma_start(
                                btk[:, bi * W:(bi + 1) * W],
                                xp_window_ap(b0 + bi, g + 4 * blk))
                        bts.append(btk)
                    stA = stpool.tile([128, half * W], f32, tag="stA")
                    stB = stpool.tile([128, half * W], f32, tag="stB")
                    for p in range(half):
                        ps = pspool.tile([128, B2 * W], f32, tag="ps")
                        blks = list(range(p // 4, min(3, (p + half + 6) // 4) + 1))
                        for ib, blk in enumerate(blks):
                            wt = wts[TIDX[(half, 4 * blk - p)]]
                            nc.tensor.matmul(
                                ps[:],
                                wt[:],
                                bts[blk][:],
                                start=(ib == 0), stop=(ib == len(blks) - 1),
                                tile_position=(0, 0))
                        nc.vector.tensor_copy(stA[:, p * W:(p + 1) * W], ps[:, 0:W])
                        nc.scalar.copy(stB[:, p * W:(p + 1) * W], ps[:, W:2 * W])
                    for bi, st in ((b0, stA), (b0 + 1, stB)):
                        dst = out_d[bi, :, g:g + 2 * half, :].rearrange(
                            "f (h p) c -> h f (p c)", h=2)
                        nc.sync.dma_start(dst, st[:])

    nc.compile()
    return nc


def _prep_host(input, kern, mode):
    xp_np = np.zeros((B_FULL, XP_ROWS, WP), np.float32)
    xp_np[:, PAD:PAD + H, PAD:PAD + W] = input[:, 0]
    wtab = _make_weight_tables(kern)
    if mode == "bf16":
        import ml_dtypes
        xp_np = xp_np.astype(ml_dtypes.bfloat16)
        wtab = wtab.astype(ml_dtypes.bfloat16)
    return xp_np, wtab


def kernel(input, kernel):
    if "nc" not in _cache:
        _cache["nc"] = _build_program(MODE)
    nc = _cache["nc"]

    input = np.ascontiguousarray(np.asarray(input, dtype=np.float32))
    kern = np.ascontiguousarray(np.asarray(kernel, dtype=np.float32))
    xp_np, wtab = _prep_host(input, kern, MODE)
    in_maps = [
        {"xp": xp_np[B_LOC * c:B_LOC * (c + 1)], "wtab": wtab}
        for c in range(N_CORES)
    ]
    res = run_bass_kernel_spmd(nc, in_maps, core_ids=list(range(N_CORES)))
    _cache["last_results"] = res
    return np.concatenate([r["out"] for r in res.results], axis=0)
